# revision 50
# baseline (speedup 1.0000x reference)
"""Trainium2 Bass kernel: segment-mean of gathered token rows + small MLP.

Computation (matches the nn_Discriminator reference):
    hid   = transformer_hidden.reshape(-1, H)          # [V, H]
    g     = hid[indices]                               # [N_IDX, H]
    h     = segment_mean(g, segment_ids, N_SEG)        # [N_SEG, H]
    z     = gelu(h @ W_dense.T + b_dense)              # [N_SEG, H]
    out   = (z @ W_proj.T + b_proj).reshape(-1)        # [N_SEG]

Sharding: data-parallel over segments; core c owns 8 blocks of 128
segments. segment_ids is sorted, so each block's tokens are a contiguous
slice of `indices`. Per block the device:
  - dma_gathers the block's token rows (fp16, int16 indices) into SBUF
  - accumulates sums[seg, :] = onehot.T @ rows on the PE (PSUM f32)
  - scales by 1/count (DVE) and PE-transposes into HT [H, segs]
then dense (PE, fp16) + erf-gelu w/ bias (ACT) + projection (PE), run as
four 256-segment waves interleaved with the gather stream.

Trace-driven tuning (133.7us -> ~126us on 8 axon trn2 cores):
  - The hard walls: ~7.5us Tile preamble + ~12us gather-ucode LOAD_LIB
    before the first dma_gather can execute, and ~75us of Q7 SWDGE
    descriptor emission (~8.7ns per gathered row, all 8 Q7 cores).
    Everything else hides under these or lands in the tail.
  - idx DMA is split: a small head slice covering block 0's first calls
    is its own DMA so the first gather is not gated on the full array.
  - All blocks' one-hots ([p, c*128+j] = (segl[p,c]==j)) are built
    upfront, one broadcast DVE compare per block, in the otherwise-idle
    window before the first gather lands.
  - Dense waves (2 blocks each) keep the PE fed mid-stream and leave
    only one 256-seg wave after the last gather; each wave DMAs its
    output slice immediately. pzt bufs=3 reduces j8-chain stalls.
  - Block 0 ramps call sizes up ([1,4,5...]), the last block tapers
    down ([...,3,2,1]) so its segment-sum pipelines with the final
    emissions.
  - Measured dead ends: xbar dma-transpose for HT (Tile serializes it
    against the SWDGE gather stream: 1.4x slower), deeper gather
    buffering (SWDGE ring-full mid-emission stalls), MAXSUB=7 call
    granularity, single_packet=False, a warm-up gather to preload the
    Q7 library (the lib load overlaps the idx DMA wait anyway).

Blocks are processed in descending-token-count order per core so the
per-position padded chunk counts (max across cores, baked into the SPMD
program) stay tight; the host un-permutes the per-block outputs.
"""

import math
import os

import numpy as np

import concourse.bacc as bacc
import concourse.bass as bass
import concourse.mybir as mybir
import concourse.tile as tile
from concourse import bass_utils
from concourse.masks import make_identity

P = 128
B, S, H = 8, 4096, 1024
V = B * S               # 32768 gatherable rows
N_IDX = 65536
N_SEG = 8192
N_CORES = 8
SEGS_PER_CORE = N_SEG // N_CORES     # 1024
NBLK = SEGS_PER_CORE // P            # 8 seg-blocks of 128 segments per core
NJ = H // P                          # 8 partition-blocks of the hidden dim
MAXSUB = 5                           # max chunks (5*128=640 idx) per dma_gather
GBUFS = 8                            # gather tile buffering depth; deeper
                                     # (12) runs the gpsimd engine into SWDGE
                                     # ring-full mid-emission stalls, and
                                     # MAXSUB=7 coarsens consumer granularity
                                     # (both measured slower)
WAVE_BLOCKS = (2, 2, 2, 2)           # dense wave sizes in 128-seg blocks

FP = mybir.dt.float32
I32 = mybir.dt.int32
I16 = mybir.dt.int16

# knobs test.py can poke (trace etc). Not used by the grading path.
RUN_KWARGS: dict = {}
LAST_RESULTS = None

DATA_DT = os.environ.get("KERNEL_DT", "float16")


_TAPER = {6: [3, 2, 1], 5: [2, 2, 1], 4: [2, 1, 1], 3: [2, 1],
          2: [1, 1], 1: [1], 0: []}


def _subs_for(b, k, nblk=NBLK):
    """Call split (chunks per dma_gather) for block position b.

    Block 0 ramps up ([1, 4, 5...]) so the first one-hot matmuls start
    ~1.5us after the idx head lands; the last block tapers down
    ([..., 3, 2, 1]) so its segment-sum matmuls pipeline with the final
    emissions instead of all landing in the post-gather tail."""
    out = []
    if b == 0 and k > 1:
        out = [1]
        k -= 1
        if k > 4:
            out.append(4)
            k -= 4
    tail = []
    if b == nblk - 1 and k > 0:
        r = min(6, k) if k > 6 or k in _TAPER else k
        tail = _TAPER[r]
        k -= r
    while k > 0:
        out.append(min(MAXSUB, k))
        k -= out[-1]
    return out + tail


def _emit(nc, cfg):
    """Emit the per-core program.

    cfg: V, K_pos (chunk count per block position), SEGS, act, dt.
    """
    cV, K_pos, SEGS = cfg["V"], cfg["K_pos"], cfg["SEGS"]
    act_fn, DT = cfg["act"], cfg["dt"]
    cNB = len(K_pos)
    CT = sum(K_pos)                       # total chunks in the stream
    # head slice: idx columns for block 0's first two calls
    head_chunks = sum(_subs_for(0, K_pos[0])[:2])

    hid = nc.dram_tensor("hid", [cV, H], DT, kind="ExternalInput").ap()
    # int16 gather indices in dma_gather wrapped layout: chunk-stream order;
    # call i covering chunks [c0, c0+sub) owns columns [c0*8, (c0+sub)*8);
    # the call's token t=col*16+p sits at partition p%16, replicated 8x down
    # the partition dim for the 8 Q7 cores.
    idxd = nc.dram_tensor("idx16", [P, CT * 8], I16, kind="ExternalInput").ap()
    segld = nc.dram_tensor("segl", [P, CT], I32, kind="ExternalInput").ap()
    invcd = nc.dram_tensor("invc", [P, cNB], FP, kind="ExternalInput").ap()
    iotad = nc.dram_tensor("iota", [P, P], I32, kind="ExternalInput").ap()
    wdtd = nc.dram_tensor("wdt", [H, H], DT, kind="ExternalInput").ap()
    bdend = nc.dram_tensor("bdense", [P, NJ], FP, kind="ExternalInput").ap()
    wptd = nc.dram_tensor("wpt", [P, NJ], DT, kind="ExternalInput").ap()
    bprojd = nc.dram_tensor("bproj", [1, 1], FP, kind="ExternalInput").ap()
    outd = nc.dram_tensor("out", [1, SEGS], FP, kind="ExternalOutput").ap()

    with tile.TileContext(nc) as tc:
        with (
            tc.tile_pool(name="const", bufs=1) as cpool,
            tc.tile_pool(name="gp", bufs=1) as gpool,
            tc.tile_pool(name="ip", bufs=1) as ipool,
            tc.tile_pool(name="wk", bufs=1) as wkpool,
            tc.tile_pool(name="pseg", bufs=1, space="PSUM") as pseg,
            tc.tile_pool(name="ptr", bufs=1, space="PSUM") as ptr,
            tc.tile_pool(name="pzt", bufs=1, space="PSUM") as pzt,
            tc.tile_pool(name="ppred", bufs=1, space="PSUM") as ppred,
        ):
            # ---- index loads on the Sync HWDGE queue. The head slice (first
            # two calls of block 0) is its own small DMA so the first
            # dma_gather is not gated on the full idx transfer. ----
            idx_all = ipool.tile([P, CT * 8], I16)
            nc.sync.dma_start(out=idx_all[:, :head_chunks * 8],
                              in_=idxd[:, :head_chunks * 8])
            nc.sync.dma_start(out=idx_all[:, head_chunks * 8:],
                              in_=idxd[:, head_chunks * 8:])

            # ---- small constants on the Scalar HWDGE queue ----
            segl_all = ipool.tile([P, CT], I32)
            nc.scalar.dma_start(out=segl_all[:], in_=segld)
            invc_all = ipool.tile([P, cNB], FP)
            nc.scalar.dma_start(out=invc_all[:], in_=invcd)
            iota_sb = cpool.tile([P, P], I32)
            nc.scalar.dma_start(out=iota_sb[:], in_=iotad)
            # ident is built lazily (after block 0's gathers are issued) so
            # its gpsimd memset/DVE ops don't sit ahead of the first
            # dma_gather in the engine queues
            ident = cpool.tile([P, P], DT)
            ident_made = [False]

            def make_ident():
                if not ident_made[0]:
                    make_identity(nc, ident[:])
                    ident_made[0] = True

            # weight loads are emitted lazily (after the first gathers) so the
            # gathers' DMA-completion lanes don't transitively wait on 2MB of
            # weight traffic
            wdt_sb = []
            bden_sb = wpt_sb = bproj_sb = None

            def load_weights():
                nonlocal bden_sb, wpt_sb, bproj_sb
                for k8 in range(NJ):
                    w = cpool.tile([P, H], DT, name=f"wdt_sb{k8}", tag=f"wdt{k8}")
                    nc.scalar.dma_start(out=w[:], in_=wdtd[k8 * P:(k8 + 1) * P, :])
                    wdt_sb.append(w)
                bden_sb = cpool.tile([P, NJ], FP)
                nc.scalar.dma_start(out=bden_sb[:], in_=bdend)
                wpt_sb = cpool.tile([P, NJ], DT)
                nc.scalar.dma_start(out=wpt_sb[:], in_=wptd)
                bproj_sb = cpool.tile([1, 1], FP)
                nc.scalar.dma_start(out=bproj_sb[:], in_=bprojd)
            # HT lives in one tile [P, NJ*SEGS]: partition p, column k*SEGS+s
            # holds h[s, k*128+p] (written by the xbar dma transpose)
            ht_all = cpool.tile([P, NJ * SEGS], DT, name="ht_all")
            zt_sb = []
            for k8 in range(NJ):
                t = cpool.tile([P, SEGS], DT, name=f"zt_sb{k8}", tag=f"zt{k8}")
                zt_sb.append(t)

            pred_sb = cpool.tile([1, SEGS], FP, name="pred_sb")

            # ---- all one-hots upfront: one DVE op per block builds
            # oh_blk[p, c*128+j] = (segl[p, off+c] == j). Only needs
            # segl+iota, so the DVE does this in the otherwise-idle window
            # before the first gather lands ----
            oh_blks = []
            off = 0
            for b in range(cNB):
                K = K_pos[b]
                ohb = cpool.tile([P, K * P], DT, name=f"ohb{b}")
                segb = segl_all[:, off:off + K].to_broadcast((P, K, P))
                ia = iota_sb[:, :]
                iota_b = bass.AP(
                    ia.tensor, ia.offset,
                    [list(ia.ap[0]), [0, K], list(ia.ap[1])])
                nc.vector.tensor_tensor(
                    out=ohb[:].rearrange("p (c j) -> p c j", j=P),
                    in0=segb, in1=iota_b,
                    op=mybir.AluOpType.is_equal,
                )
                oh_blks.append(ohb)
                off += K

            def dense_wave(w0, nsegs):
                """dense+gelu+proj for segments [w0, w0+nsegs)."""
                for j8 in range(NJ):
                    pz = pzt.tile([P, nsegs], FP, name=f"pz{w0}_{j8}",
                                  tag="pzt", bufs=3, padded_shape=[P, 512])
                    for k8 in range(NJ):
                        nc.tensor.matmul(
                            out=pz[:],
                            lhsT=wdt_sb[k8][:, j8 * P:(j8 + 1) * P],
                            rhs=ht_all[:, k8 * SEGS + w0:k8 * SEGS + w0 + nsegs],
                            start=(k8 == 0),
                            stop=(k8 == NJ - 1),
                        )
                    nc.scalar.activation(
                        out=zt_sb[j8][:, w0:w0 + nsegs],
                        in_=pz[:],
                        func=act_fn,
                        bias=bden_sb[:, j8:j8 + 1],
                        scale=1.0,
                    )
                ppd = ppred.tile([1, nsegs], FP, name=f"ppd{w0}", tag="ppred",
                                 bufs=1, padded_shape=[1, 512])
                for j8 in range(NJ):
                    nc.tensor.matmul(
                        out=ppd[:],
                        lhsT=wpt_sb[:, j8:j8 + 1],
                        rhs=zt_sb[j8][:, w0:w0 + nsegs],
                        start=(j8 == 0),
                        stop=(j8 == NJ - 1),
                    )
                nc.vector.tensor_scalar_add(
                    out=pred_sb[0:1, w0:w0 + nsegs], in0=ppd[0:1, :],
                    scalar1=bproj_sb[0:1, 0:1])
                # ship this wave's slice immediately; the final DMA's
                # write-receipt latency then only covers the last slice
                nc.sync.dma_start(out=outd[:, w0:w0 + nsegs],
                                  in_=pred_sb[0:1, w0:w0 + nsegs])

            # ---- the chunk-stream gather + per block: segment-sum +
            # transpose; tapered dense waves keep the post-gather tail to a
            # single 128-seg wave ----
            gts = {}          # global chunk idx -> (tile, local col)
            c0 = 0            # global chunk cursor (emitted gather calls)
            call_i = 0
            # wave emission is deferred one block past its data so a
            # late-arriving block's segment-sum is not queued behind a
            # wave's matmuls on the in-order PE queue
            fire_map = {}
            blk_acc = 0
            w0 = 0
            for nb in WAVE_BLOCKS:
                blk_acc += nb
                fire_map[min(blk_acc + 1, cNB)] = (w0, nb * P)
                w0 += nb * P

            off_b = 0
            for b in range(cNB):
                K = K_pos[b]
                # emit this block's gather calls
                for sub in _subs_for(b, K, cNB):
                    g = gpool.tile([P, sub * H], DT, name=f"g{call_i}",
                                   tag="g", bufs=GBUFS,
                                   padded_shape=[P, MAXSUB * H])
                    nc.gpsimd.dma_gather(
                        out_ap=g[:].rearrange("p (s e) -> p s e", e=H),
                        in_ap=hid,
                        idxs_ap=idx_all[:, c0 * 8:(c0 + sub) * 8],
                        num_idxs=sub * P,
                        num_idxs_reg=sub * P,
                        elem_size=H,
                    )
                    for cl in range(sub):
                        gts[c0 + cl] = (g, cl)
                    c0 += sub
                    call_i += 1
                    if call_i == 4:
                        load_weights()

                ps = pseg.tile([P, H], FP, name=f"ps{b}", tag="pseg", bufs=1)
                for c in range(K):
                    g, cl = gts[off_b + c]
                    for hf in range(2):
                        nc.tensor.matmul(
                            out=ps[:, hf * 512:(hf + 1) * 512],
                            lhsT=oh_blks[b][:, c * P:(c + 1) * P],
                            rhs=g[:, cl * H + hf * 512: cl * H + (hf + 1) * 512],
                            start=(c == 0),
                            stop=(c == K - 1),
                        )

                # scale in two halves so the first transposes start after
                # half the PSUM is drained, not all of it. (An xbar
                # dma-transpose would free the PE here, but Tile serializes
                # DMA_TRANSPOSE against the SWDGE gather stream -> 1.35x
                # slower overall. PE transposes it is.)
                make_ident()
                hbh = []
                for h2 in range(2):
                    hb = wkpool.tile([P, H // 2], DT, name=f"hb{b}_{h2}",
                                     tag="hb", bufs=4)
                    nc.vector.tensor_scalar_mul(
                        out=hb[:], in0=ps[:, h2 * 512:(h2 + 1) * 512],
                        scalar1=invc_all[:, b:b + 1])
                    hbh.append(hb)
                for k8 in range(NJ):
                    hb = hbh[k8 // 4]
                    pt = ptr.tile([P, P], DT, name=f"pt{b}_{k8}", tag="ptr",
                                  bufs=2)
                    nc.tensor.transpose(
                        out=pt[:], in_=hb[:, (k8 % 4) * P:(k8 % 4 + 1) * P],
                        identity=ident[:]
                    )
                    nc.vector.tensor_copy(
                        out=ht_all[:, k8 * SEGS + b * P:k8 * SEGS + (b + 1) * P],
                        in_=pt[:]
                    )

                off_b += K
                if b + 1 in fire_map:
                    dense_wave(*fire_map[b + 1])
    return nc


_CACHE: dict = {}


def build(cfg_key):
    """cfg_key: (V, K_pos tuple, SEGS, act_name, dt_name). Returns compiled nc."""
    if cfg_key in _CACHE:
        return _CACHE[cfg_key]
    cV, K_pos, SEGS, act_name, dt_name = cfg_key
    cfg = {
        "V": cV, "K_pos": list(K_pos), "SEGS": SEGS,
        "act": getattr(mybir.ActivationFunctionType, act_name),
        "dt": getattr(mybir.dt, dt_name),
    }
    nc = bacc.Bacc("TRN2", target_bir_lowering=False, debug=False,
                   enable_asserts=False)
    _emit(nc, cfg)
    nc.compile()
    _CACHE[cfg_key] = nc
    return nc


def plan_blocks(seg_all, n_seg):
    """Assign 128-segment blocks to cores with an LPT-style snake over the
    descending-count order: position p of core c gets the (p*n_cores + c)-th
    (or boustrophedon-reversed) largest block. Cores stay balanced and the
    per-position chunk count (max across cores, baked into the SPMD program)
    stays tight. Returns (bounds, order[n_cores, nblk], K_pos[nblk])."""
    n_blocks = n_seg // P
    n_cores = N_CORES if n_seg == N_SEG else 1
    nblk = n_blocks // n_cores
    bounds = np.searchsorted(seg_all, np.arange(0, n_seg + P, P))
    cnts = np.diff(bounds)
    srt = np.argsort(-cnts, kind="stable")
    order = np.empty((n_cores, nblk), np.int64)
    for p in range(nblk):
        grp = srt[p * n_cores:(p + 1) * n_cores]
        order[:, p] = grp if p % 2 == 0 else grp[::-1]
    pos_max = cnts[order].max(axis=0)            # [nblk]
    K_pos = np.maximum(1, np.ceil(pos_max / P).astype(int))
    return bounds, order, K_pos


def prep_core_inputs(idx_all, seg_all, bounds, order, K_pos):
    """Host-side shard prep. Returns per-core input dict list."""
    nblk = order.shape[1]
    CT = int(np.sum(K_pos))
    counts = np.bincount(seg_all, minlength=int(order.max() + 1) * P
                         ).astype(np.float64)
    invc_all = (1.0 / np.maximum(counts, 1.0)).astype(np.float32)
    per_core = []
    for c in range(order.shape[0]):
        idx16_host = np.zeros((P, CT * 8), np.int16)
        segl_host = np.full((P, CT), -1, np.int32)
        invc_host = np.ones((P, nblk), np.float32)
        off_b = 0
        call_i = 0
        for b in range(nblk):
            gb = int(order[c, b])
            k = int(K_pos[b])
            t0, t1 = bounds[gb], bounds[gb + 1]
            rows = idx_all[t0:t1]
            locs = (seg_all[t0:t1] - gb * P).astype(np.int32)
            o = np.argsort(rows, kind="stable")
            rows, locs = rows[o], locs[o]
            n = rows.shape[0]
            assert n <= k * P, f"block {gb} has {n} tokens > capacity {k * P}"
            lpad = np.full(k * P, -1, np.int32)
            lpad[:n] = locs
            rpad = np.zeros(k * P, np.int32)
            rpad[:n] = rows
            # matmul chunk j reads token t = j*P + p at gather slot (p, j)
            segl_host[:, off_b:off_b + k] = lpad.reshape(k, P).T
            c0 = 0
            for sub in _subs_for(b, k):
                tl = rpad[c0 * P:(c0 + sub) * P]
                wrapped = tl.reshape(sub * 8, 16).T.astype(np.int16)
                idx16_host[:, (off_b + c0) * 8:(off_b + c0 + sub) * 8] = \
                    np.tile(wrapped, (8, 1))
                c0 += sub
                call_i += 1
            invc_host[:, b] = invc_all[gb * P:(gb + 1) * P]
            off_b += k
        per_core.append({"idx16": idx16_host, "segl": segl_host,
                         "invc": invc_host})
    return per_core


def kernel(transformer_hidden, indices, segment_ids, W_dense, b_dense,
           W_proj, b_proj):
    global LAST_RESULTS
    np_dt = np.float16 if DATA_DT == "float16" else np.float32
    hid = np.ascontiguousarray(
        np.asarray(transformer_hidden, np.float32).reshape(V, H).astype(np_dt))
    idx_all = np.asarray(indices, np.int32).reshape(-1)
    seg_all = np.asarray(segment_ids, np.int32).reshape(-1)
    wdt = np.ascontiguousarray(np.asarray(W_dense, np.float32).T.astype(np_dt))
    bden = np.ascontiguousarray(
        np.asarray(b_dense, np.float32).reshape(NJ, P).T)
    wpt = np.ascontiguousarray(
        np.asarray(W_proj, np.float32).reshape(NJ, P).T.astype(np_dt))
    bproj = np.asarray(b_proj, np.float32).reshape(1, 1)
    iota = np.ascontiguousarray(
        np.broadcast_to(np.arange(P, dtype=np.int32), (P, P)))

    bounds, order, K_pos = plan_blocks(seg_all, N_SEG)
    per_core = prep_core_inputs(idx_all, seg_all, bounds, order, K_pos)
    shared = {"hid": hid, "iota": iota, "wdt": wdt, "bdense": bden,
              "wpt": wpt, "bproj": bproj}
    in_maps = [dict(shared, **pc) for pc in per_core]

    nc = build((V, tuple(int(x) for x in K_pos), SEGS_PER_CORE, "Gelu", DATA_DT))
    res = bass_utils.run_bass_kernel_spmd(
        nc, in_maps, core_ids=list(range(N_CORES)), **RUN_KWARGS)
    LAST_RESULTS = res

    out = np.empty(N_SEG, np.float32)
    for c in range(N_CORES):
        oc = np.asarray(res.results[c]["out"]).reshape(NBLK, P)
        for b in range(NBLK):
            gb = int(order[c, b])
            out[gb * P:(gb + 1) * P] = oc[b]
    return out


# revision 51
# speedup vs baseline: 1.1325x; 1.1325x over previous
"""Trainium2 Bass kernel: segment-mean of gathered token rows + small MLP.

Computation (matches the nn_Discriminator reference):
    hid   = transformer_hidden.reshape(-1, H)          # [V, H]
    g     = hid[indices]                               # [N_IDX, H]
    h     = segment_mean(g, segment_ids, N_SEG)        # [N_SEG, H]
    z     = gelu(h @ W_dense.T + b_dense)              # [N_SEG, H]
    out   = (z @ W_proj.T + b_proj).reshape(-1)        # [N_SEG]

Sharding: data-parallel over segments; core c owns 8 blocks of 128
segments. segment_ids is sorted, so each block's tokens are a contiguous
slice of `indices`. Per block the device:
  - dma_gathers the block's token rows (fp16, int16 indices) into SBUF
  - accumulates sums[seg, :] = onehot.T @ rows on the PE (PSUM f32)
  - scales by 1/count (DVE) and PE-transposes into HT [H, segs]
then dense (PE, fp16) + erf-gelu w/ bias (ACT) + projection (PE), run as
four 256-segment waves interleaved with the gather stream.

Trace-driven tuning (133.7us -> ~126us on 8 axon trn2 cores):
  - The hard walls: ~7.5us Tile preamble + ~12us gather-ucode LOAD_LIB
    before the first dma_gather can execute, and ~75us of Q7 SWDGE
    descriptor emission (~8.7ns per gathered row, all 8 Q7 cores).
    Everything else hides under these or lands in the tail.
  - idx DMA is split: a small head slice covering block 0's first calls
    is its own DMA so the first gather is not gated on the full array.
  - All blocks' one-hots ([p, c*128+j] = (segl[p,c]==j)) are built
    upfront, one broadcast DVE compare per block, in the otherwise-idle
    window before the first gather lands.
  - Dense waves (2 blocks each) keep the PE fed mid-stream and leave
    only one 256-seg wave after the last gather; each wave DMAs its
    output slice immediately. pzt bufs=3 reduces j8-chain stalls.
  - Block 0 ramps call sizes up ([1,4,5...]), the last block tapers
    down ([...,3,2,1]) so its segment-sum pipelines with the final
    emissions.
  - Measured dead ends: xbar dma-transpose for HT (Tile serializes it
    against the SWDGE gather stream: 1.4x slower), deeper gather
    buffering (SWDGE ring-full mid-emission stalls), MAXSUB=7 call
    granularity, single_packet=False, a warm-up gather to preload the
    Q7 library (the lib load overlaps the idx DMA wait anyway).

Blocks are processed in descending-token-count order per core so the
per-position padded chunk counts (max across cores, baked into the SPMD
program) stay tight; the host un-permutes the per-block outputs.
"""

import math
import os

import numpy as np

import concourse.bacc as bacc
import concourse.bass as bass
import concourse.mybir as mybir
import concourse.tile as tile
from concourse import bass_utils
from concourse.masks import make_identity

P = 128
B, S, H = 8, 4096, 1024
V = B * S               # 32768 gatherable rows
N_IDX = 65536
N_SEG = 8192
N_CORES = 8
SEGS_PER_CORE = N_SEG // N_CORES     # 1024
NBLK = SEGS_PER_CORE // P            # 8 seg-blocks of 128 segments per core
NJ = H // P                          # 8 partition-blocks of the hidden dim
MAXSUB = 5                           # max chunks (5*128=640 idx) per dma_gather
GBUFS = 8                            # gather tile buffering depth; deeper
                                     # (12) runs the gpsimd engine into SWDGE
                                     # ring-full mid-emission stalls, and
                                     # MAXSUB=7 coarsens consumer granularity
                                     # (both measured slower)
WAVE_BLOCKS = (2, 2, 2, 2)           # dense wave sizes in 128-seg blocks

FP = mybir.dt.float32
I32 = mybir.dt.int32
I16 = mybir.dt.int16

# knobs test.py can poke (trace etc). Not used by the grading path.
RUN_KWARGS: dict = {}
LAST_RESULTS = None

DATA_DT = os.environ.get("KERNEL_DT", "float16")


_TAPER = {6: [3, 2, 1], 5: [2, 2, 1], 4: [2, 1, 1], 3: [2, 1],
          2: [1, 1], 1: [1], 0: []}


def _subs_for(b, k, nblk=NBLK):
    """Call split (chunks per dma_gather) for block position b.

    Block 0 ramps up ([1, 4, 5...]) so the first one-hot matmuls start
    ~1.5us after the idx head lands; the last block tapers down
    ([..., 3, 2, 1]) so its segment-sum matmuls pipeline with the final
    emissions instead of all landing in the post-gather tail."""
    out = []
    if b == 0 and k > 1:
        out = [1]
        k -= 1
        if k > 4:
            out.append(4)
            k -= 4
    tail = []
    if b == nblk - 1 and k > 0:
        r = min(6, k) if k > 6 or k in _TAPER else k
        tail = _TAPER[r]
        k -= r
    while k > 0:
        out.append(min(MAXSUB, k))
        k -= out[-1]
    return out + tail


def _emit(nc, cfg):
    """Emit the per-core program.

    cfg: V, K_pos (chunk count per block position), SEGS, act, dt.
    """
    cV, K_pos, SEGS = cfg["V"], cfg["K_pos"], cfg["SEGS"]
    act_fn, DT = cfg["act"], cfg["dt"]
    cNB = len(K_pos)
    CT = sum(K_pos)                       # total chunks in the stream
    # head slice: idx columns for block 0's first two calls
    head_chunks = sum(_subs_for(0, K_pos[0])[:2])

    hid = nc.dram_tensor("hid", [cV, H], DT, kind="ExternalInput").ap()
    # int16 gather indices in dma_gather wrapped layout: chunk-stream order;
    # call i covering chunks [c0, c0+sub) owns columns [c0*8, (c0+sub)*8);
    # the call's token t=col*16+p sits at partition p%16, replicated 8x down
    # the partition dim for the 8 Q7 cores.
    idxd = nc.dram_tensor("idx16", [P, CT * 8], I16, kind="ExternalInput").ap()
    segld = nc.dram_tensor("segl", [P, CT], I32, kind="ExternalInput").ap()
    invcd = nc.dram_tensor("invc", [P, cNB], FP, kind="ExternalInput").ap()
    iotad = nc.dram_tensor("iota", [P, P], I32, kind="ExternalInput").ap()
    wdtd = nc.dram_tensor("wdt", [H, H], DT, kind="ExternalInput").ap()
    bdend = nc.dram_tensor("bdense", [P, NJ], FP, kind="ExternalInput").ap()
    wptd = nc.dram_tensor("wpt", [P, NJ], DT, kind="ExternalInput").ap()
    bprojd = nc.dram_tensor("bproj", [1, 1], FP, kind="ExternalInput").ap()
    outd = nc.dram_tensor("out", [1, SEGS], FP, kind="ExternalOutput").ap()

    with tile.TileContext(nc) as tc:
        with (
            tc.tile_pool(name="const", bufs=1) as cpool,
            tc.tile_pool(name="gp", bufs=1) as gpool,
            tc.tile_pool(name="ip", bufs=1) as ipool,
            tc.tile_pool(name="wk", bufs=1) as wkpool,
            tc.tile_pool(name="pseg", bufs=1, space="PSUM") as pseg,
            tc.tile_pool(name="ptr", bufs=1, space="PSUM") as ptr,
            tc.tile_pool(name="pzt", bufs=1, space="PSUM") as pzt,
            tc.tile_pool(name="ppred", bufs=1, space="PSUM") as ppred,
        ):
            # ---- index loads on the Sync HWDGE queue. The head slice (first
            # two calls of block 0) is its own small DMA so the first
            # dma_gather is not gated on the full idx transfer. ----
            idx_all = ipool.tile([P, CT * 8], I16)
            nc.sync.dma_start(out=idx_all[:, :head_chunks * 8],
                              in_=idxd[:, :head_chunks * 8])
            nc.sync.dma_start(out=idx_all[:, head_chunks * 8:],
                              in_=idxd[:, head_chunks * 8:])

            # ---- small constants on the Scalar HWDGE queue ----
            segl_all = ipool.tile([P, CT], I32)
            nc.scalar.dma_start(out=segl_all[:], in_=segld)
            invc_all = ipool.tile([P, cNB], FP)
            nc.scalar.dma_start(out=invc_all[:], in_=invcd)
            iota_sb = cpool.tile([P, P], I32)
            nc.scalar.dma_start(out=iota_sb[:], in_=iotad)
            # ident is built lazily (after block 0's gathers are issued) so
            # its gpsimd memset/DVE ops don't sit ahead of the first
            # dma_gather in the engine queues
            ident = cpool.tile([P, P], DT)
            ident_made = [False]

            def make_ident():
                if not ident_made[0]:
                    make_identity(nc, ident[:])
                    ident_made[0] = True

            # weight loads are emitted lazily (after the first gathers) so the
            # gathers' DMA-completion lanes don't transitively wait on 2MB of
            # weight traffic
            wdt_sb = []
            bden_sb = wpt_sb = bproj_sb = None

            def load_weights():
                nonlocal bden_sb, wpt_sb, bproj_sb
                for k8 in range(NJ):
                    w = cpool.tile([P, H], DT, name=f"wdt_sb{k8}", tag=f"wdt{k8}")
                    nc.scalar.dma_start(out=w[:], in_=wdtd[k8 * P:(k8 + 1) * P, :])
                    wdt_sb.append(w)
                bden_sb = cpool.tile([P, NJ], FP)
                nc.scalar.dma_start(out=bden_sb[:], in_=bdend)
                wpt_sb = cpool.tile([P, NJ], DT)
                nc.scalar.dma_start(out=wpt_sb[:], in_=wptd)
                bproj_sb = cpool.tile([1, 1], FP)
                nc.scalar.dma_start(out=bproj_sb[:], in_=bprojd)
            # HT lives in one tile [P, NJ*SEGS]: partition p, column k*SEGS+s
            # holds h[s, k*128+p] (written by the xbar dma transpose)
            ht_all = cpool.tile([P, NJ * SEGS], DT, name="ht_all")
            zt_sb = []
            for k8 in range(NJ):
                t = cpool.tile([P, SEGS], DT, name=f"zt_sb{k8}", tag=f"zt{k8}")
                zt_sb.append(t)

            pred_sb = cpool.tile([1, SEGS], FP, name="pred_sb")

            # ---- all one-hots upfront: one DVE op per block builds
            # oh_blk[p, c*128+j] = (segl[p, off+c] == j). Only needs
            # segl+iota, so the DVE does this in the otherwise-idle window
            # before the first gather lands ----
            oh_blks = []
            off = 0
            for b in range(cNB):
                K = K_pos[b]
                ohb = cpool.tile([P, K * P], DT, name=f"ohb{b}")
                segb = segl_all[:, off:off + K].to_broadcast((P, K, P))
                ia = iota_sb[:, :]
                iota_b = bass.AP(
                    ia.tensor, ia.offset,
                    [list(ia.ap[0]), [0, K], list(ia.ap[1])])
                nc.vector.tensor_tensor(
                    out=ohb[:].rearrange("p (c j) -> p c j", j=P),
                    in0=segb, in1=iota_b,
                    op=mybir.AluOpType.is_equal,
                )
                oh_blks.append(ohb)
                off += K

            def dense_wave(w0, nsegs):
                """dense+gelu+proj for segments [w0, w0+nsegs)."""
                for j8 in range(NJ):
                    pz = pzt.tile([P, nsegs], FP, name=f"pz{w0}_{j8}",
                                  tag="pzt", bufs=3, padded_shape=[P, 512])
                    for k8 in range(NJ):
                        nc.tensor.matmul(
                            out=pz[:],
                            lhsT=wdt_sb[k8][:, j8 * P:(j8 + 1) * P],
                            rhs=ht_all[:, k8 * SEGS + w0:k8 * SEGS + w0 + nsegs],
                            start=(k8 == 0),
                            stop=(k8 == NJ - 1),
                        )
                    nc.scalar.activation(
                        out=zt_sb[j8][:, w0:w0 + nsegs],
                        in_=pz[:],
                        func=act_fn,
                        bias=bden_sb[:, j8:j8 + 1],
                        scale=1.0,
                    )
                ppd = ppred.tile([1, nsegs], FP, name=f"ppd{w0}", tag="ppred",
                                 bufs=1, padded_shape=[1, 512])
                for j8 in range(NJ):
                    nc.tensor.matmul(
                        out=ppd[:],
                        lhsT=wpt_sb[:, j8:j8 + 1],
                        rhs=zt_sb[j8][:, w0:w0 + nsegs],
                        start=(j8 == 0),
                        stop=(j8 == NJ - 1),
                    )
                nc.vector.tensor_scalar_add(
                    out=pred_sb[0:1, w0:w0 + nsegs], in0=ppd[0:1, :],
                    scalar1=bproj_sb[0:1, 0:1])
                # ship this wave's slice immediately; the final DMA's
                # write-receipt latency then only covers the last slice
                nc.sync.dma_start(out=outd[:, w0:w0 + nsegs],
                                  in_=pred_sb[0:1, w0:w0 + nsegs])

            # ---- the chunk-stream gather + per block: segment-sum +
            # transpose; tapered dense waves keep the post-gather tail to a
            # single 128-seg wave ----
            gts = {}          # global chunk idx -> (tile, local col)
            c0 = 0            # global chunk cursor (emitted gather calls)
            call_i = 0
            # each wave fires right after its last block's transposes
            # (deferring waves one block was measured 14% slower: waves
            # then delay the following block's segment-sum on the
            # in-order PE queue and the gather stalls on tile frees)
            fire_map = {}
            blk_acc = 0
            w0 = 0
            for nb in WAVE_BLOCKS:
                blk_acc += nb
                fire_map[blk_acc] = (w0, nb * P)
                w0 += nb * P

            off_b = 0
            for b in range(cNB):
                K = K_pos[b]
                # emit this block's gather calls
                for sub in _subs_for(b, K, cNB):
                    g = gpool.tile([P, sub * H], DT, name=f"g{call_i}",
                                   tag="g", bufs=GBUFS,
                                   padded_shape=[P, MAXSUB * H])
                    nc.gpsimd.dma_gather(
                        out_ap=g[:].rearrange("p (s e) -> p s e", e=H),
                        in_ap=hid,
                        idxs_ap=idx_all[:, c0 * 8:(c0 + sub) * 8],
                        num_idxs=sub * P,
                        num_idxs_reg=sub * P,
                        elem_size=H,
                    )
                    for cl in range(sub):
                        gts[c0 + cl] = (g, cl)
                    c0 += sub
                    call_i += 1
                    if call_i == 4:
                        load_weights()

                ps = pseg.tile([P, H], FP, name=f"ps{b}", tag="pseg", bufs=1)
                for c in range(K):
                    g, cl = gts[off_b + c]
                    for hf in range(2):
                        nc.tensor.matmul(
                            out=ps[:, hf * 512:(hf + 1) * 512],
                            lhsT=oh_blks[b][:, c * P:(c + 1) * P],
                            rhs=g[:, cl * H + hf * 512: cl * H + (hf + 1) * 512],
                            start=(c == 0),
                            stop=(c == K - 1),
                        )

                # scale in two halves so the first transposes start after
                # half the PSUM is drained, not all of it. (An xbar
                # dma-transpose would free the PE here, but Tile serializes
                # DMA_TRANSPOSE against the SWDGE gather stream -> 1.35x
                # slower overall. PE transposes it is.)
                make_ident()
                hbh = []
                for h2 in range(2):
                    hb = wkpool.tile([P, H // 2], DT, name=f"hb{b}_{h2}",
                                     tag="hb", bufs=4)
                    nc.vector.tensor_scalar_mul(
                        out=hb[:], in0=ps[:, h2 * 512:(h2 + 1) * 512],
                        scalar1=invc_all[:, b:b + 1])
                    hbh.append(hb)
                for k8 in range(NJ):
                    hb = hbh[k8 // 4]
                    pt = ptr.tile([P, P], DT, name=f"pt{b}_{k8}", tag="ptr",
                                  bufs=2)
                    nc.tensor.transpose(
                        out=pt[:], in_=hb[:, (k8 % 4) * P:(k8 % 4 + 1) * P],
                        identity=ident[:]
                    )
                    nc.vector.tensor_copy(
                        out=ht_all[:, k8 * SEGS + b * P:k8 * SEGS + (b + 1) * P],
                        in_=pt[:]
                    )

                off_b += K
                if b + 1 in fire_map:
                    dense_wave(*fire_map[b + 1])
    return nc


_CACHE: dict = {}


def build(cfg_key):
    """cfg_key: (V, K_pos tuple, SEGS, act_name, dt_name). Returns compiled nc."""
    if cfg_key in _CACHE:
        return _CACHE[cfg_key]
    cV, K_pos, SEGS, act_name, dt_name = cfg_key
    cfg = {
        "V": cV, "K_pos": list(K_pos), "SEGS": SEGS,
        "act": getattr(mybir.ActivationFunctionType, act_name),
        "dt": getattr(mybir.dt, dt_name),
    }
    nc = bacc.Bacc("TRN2", target_bir_lowering=False, debug=False,
                   enable_asserts=False)
    _emit(nc, cfg)
    nc.compile()
    _CACHE[cfg_key] = nc
    return nc


def plan_blocks(seg_all, n_seg):
    """Assign 128-segment blocks to cores with an LPT-style snake over the
    descending-count order: position p of core c gets the (p*n_cores + c)-th
    (or boustrophedon-reversed) largest block. Cores stay balanced and the
    per-position chunk count (max across cores, baked into the SPMD program)
    stays tight. Returns (bounds, order[n_cores, nblk], K_pos[nblk])."""
    n_blocks = n_seg // P
    n_cores = N_CORES if n_seg == N_SEG else 1
    nblk = n_blocks // n_cores
    bounds = np.searchsorted(seg_all, np.arange(0, n_seg + P, P))
    cnts = np.diff(bounds)
    srt = np.argsort(-cnts, kind="stable")
    order = np.empty((n_cores, nblk), np.int64)
    for p in range(nblk):
        grp = srt[p * n_cores:(p + 1) * n_cores]
        order[:, p] = grp if p % 2 == 0 else grp[::-1]
    pos_max = cnts[order].max(axis=0)            # [nblk]
    K_pos = np.maximum(1, np.ceil(pos_max / P).astype(int))
    return bounds, order, K_pos


def prep_core_inputs(idx_all, seg_all, bounds, order, K_pos):
    """Host-side shard prep. Returns per-core input dict list."""
    nblk = order.shape[1]
    CT = int(np.sum(K_pos))
    counts = np.bincount(seg_all, minlength=int(order.max() + 1) * P
                         ).astype(np.float64)
    invc_all = (1.0 / np.maximum(counts, 1.0)).astype(np.float32)
    per_core = []
    for c in range(order.shape[0]):
        idx16_host = np.zeros((P, CT * 8), np.int16)
        segl_host = np.full((P, CT), -1, np.int32)
        invc_host = np.ones((P, nblk), np.float32)
        off_b = 0
        call_i = 0
        for b in range(nblk):
            gb = int(order[c, b])
            k = int(K_pos[b])
            t0, t1 = bounds[gb], bounds[gb + 1]
            rows = idx_all[t0:t1]
            locs = (seg_all[t0:t1] - gb * P).astype(np.int32)
            o = np.argsort(rows, kind="stable")
            rows, locs = rows[o], locs[o]
            n = rows.shape[0]
            assert n <= k * P, f"block {gb} has {n} tokens > capacity {k * P}"
            lpad = np.full(k * P, -1, np.int32)
            lpad[:n] = locs
            rpad = np.zeros(k * P, np.int32)
            rpad[:n] = rows
            # matmul chunk j reads token t = j*P + p at gather slot (p, j)
            segl_host[:, off_b:off_b + k] = lpad.reshape(k, P).T
            c0 = 0
            for sub in _subs_for(b, k):
                tl = rpad[c0 * P:(c0 + sub) * P]
                wrapped = tl.reshape(sub * 8, 16).T.astype(np.int16)
                idx16_host[:, (off_b + c0) * 8:(off_b + c0 + sub) * 8] = \
                    np.tile(wrapped, (8, 1))
                c0 += sub
                call_i += 1
            invc_host[:, b] = invc_all[gb * P:(gb + 1) * P]
            off_b += k
        per_core.append({"idx16": idx16_host, "segl": segl_host,
                         "invc": invc_host})
    return per_core


def kernel(transformer_hidden, indices, segment_ids, W_dense, b_dense,
           W_proj, b_proj):
    global LAST_RESULTS
    np_dt = np.float16 if DATA_DT == "float16" else np.float32
    hid = np.ascontiguousarray(
        np.asarray(transformer_hidden, np.float32).reshape(V, H).astype(np_dt))
    idx_all = np.asarray(indices, np.int32).reshape(-1)
    seg_all = np.asarray(segment_ids, np.int32).reshape(-1)
    wdt = np.ascontiguousarray(np.asarray(W_dense, np.float32).T.astype(np_dt))
    bden = np.ascontiguousarray(
        np.asarray(b_dense, np.float32).reshape(NJ, P).T)
    wpt = np.ascontiguousarray(
        np.asarray(W_proj, np.float32).reshape(NJ, P).T.astype(np_dt))
    bproj = np.asarray(b_proj, np.float32).reshape(1, 1)
    iota = np.ascontiguousarray(
        np.broadcast_to(np.arange(P, dtype=np.int32), (P, P)))

    bounds, order, K_pos = plan_blocks(seg_all, N_SEG)
    per_core = prep_core_inputs(idx_all, seg_all, bounds, order, K_pos)
    shared = {"hid": hid, "iota": iota, "wdt": wdt, "bdense": bden,
              "wpt": wpt, "bproj": bproj}
    in_maps = [dict(shared, **pc) for pc in per_core]

    nc = build((V, tuple(int(x) for x in K_pos), SEGS_PER_CORE, "Gelu", DATA_DT))
    res = bass_utils.run_bass_kernel_spmd(
        nc, in_maps, core_ids=list(range(N_CORES)), **RUN_KWARGS)
    LAST_RESULTS = res

    out = np.empty(N_SEG, np.float32)
    for c in range(N_CORES):
        oc = np.asarray(res.results[c]["out"]).reshape(NBLK, P)
        for b in range(NBLK):
            gb = int(order[c, b])
            out[gb * P:(gb + 1) * P] = oc[b]
    return out
